# revision 8
# baseline (speedup 1.0000x reference)
"""BertSelfAttention (relu-softmax variant) on 8 TRN2 NeuronCores.

Sharding: data-parallel over batch (B=2) x tensor-parallel over head groups
(16 heads -> 4 groups of 4). Core c handles batch c//4, heads 4*(c%4)..4*(c%4)+3.
Each core computes its [S, 256] slice of the context output; the host
concatenates slices. No cross-core collectives needed.

Per-core math (S=2048, H=1024, 4 local heads of dim 64):
  XT[j, s]   = X^T                       (PE transposes, fp32)
  QT[d, s]   = (Wq_h @ XT) * 0.125 + bq  (d on partitions, 2 heads packed per 128)
  KT[d, s]   =  Wk_h @ XT + bk
  V[s, d]    =  X @ Wv_h^T + bv          (natural layout, +ones column per head)
  ST[k, q]   =  KT^T-slice . QT          (scores^T, row-packed pairs of heads)
  RT[k, q]   =  relu(ST + mask[k])       (ACT / DVE alternating)
  CU^T[d', q] = V_aug^T-slice . RT       (d'=65: 64 ctx dims + denominator row)
  out[q, d]  = transpose(CU^T) rows / (denom + eps)

Matmuls run as float32r (full-rate fp32 streaming; moving dim >= 256).
"""

import numpy as np

import concourse.bacc as bacc
import concourse.bass as bass
import concourse.tile as tile
from concourse import mybir
from concourse import bass_utils
from concourse.masks import make_identity

F32 = mybir.dt.float32
F32R = mybir.dt.float32r
AF = mybir.ActivationFunctionType
ALU = mybir.AluOpType

B, S, H = 2, 2048, 1024
NH_CORE = 4          # heads per core
D = 64               # head dim
DC = NH_CORE * D     # 256 output dims per core
EPS = 1e-12
SCALE = 1.0 / 8.0    # 1/sqrt(64)

JT = H // 128        # 8 j-tiles (contraction tiles for projections)
ST_T = S // 128      # 16 s-tiles
QC = S // 512        # 4 q-chunks
KT_T = S // 128      # 16 k-tiles

_CACHE = {}


def _build():
    nc = bacc.Bacc("TRN2", target_bir_lowering=False, debug=False)

    x_d = nc.dram_tensor("x", [S, H], F32, kind="ExternalInput")
    wq_d = nc.dram_tensor("wq", [DC, H], F32, kind="ExternalInput")
    wk_d = nc.dram_tensor("wk", [DC, H], F32, kind="ExternalInput")
    wv_d = nc.dram_tensor("wv", [DC, H], F32, kind="ExternalInput")
    bq_d = nc.dram_tensor("bq", [DC], F32, kind="ExternalInput")
    bk_d = nc.dram_tensor("bk", [DC], F32, kind="ExternalInput")
    bv_d = nc.dram_tensor("bv", [DC], F32, kind="ExternalInput")
    m_d = nc.dram_tensor("mask", [S], F32, kind="ExternalInput")
    out_d = nc.dram_tensor("out", [S, DC], F32, kind="ExternalOutput")

    with tile.TileContext(nc) as tc:
        with tc.tile_pool(name="const", bufs=1) as consts, \
             tc.tile_pool(name="big", bufs=1) as big:
            ident = consts.tile([128, 128], F32)
            make_identity(nc, ident[:])

            # --- small constants ---------------------------------------
            # mask: [S] -> mask_sb[p, kt] = mask[kt*128 + p]
            m_nat = consts.tile([KT_T, 128], F32)
            nc.sync.dma_start(m_nat[:], m_d.ap().rearrange("(a b) -> a b", b=128))
            mask_sb = consts.tile([128, KT_T], F32)
            with tc.tile_pool(name="ps_m", bufs=1, space="PSUM") as ps_m:
                pm = ps_m.tile([128, KT_T], F32)
                nc.tensor.transpose(pm[:], m_nat[:], ident[:KT_T, :KT_T])
                nc.vector.tensor_copy(mask_sb[:], pm[:])

            # biases: [256] -> [p, hp] per-partition layout for Q/K psum copy
            bq_sb = consts.tile([128, 2], F32)
            nc.sync.dma_start(bq_sb[:], bq_d.ap().rearrange("(h p) -> p h", p=128))
            nc.vector.tensor_scalar_mul(bq_sb[:], bq_sb[:], SCALE)
            bk_sb = consts.tile([128, 2], F32)
            nc.sync.dma_start(bk_sb[:], bk_d.ap().rearrange("(h p) -> p h", p=128))
            # bv broadcast across partitions: [128, 4, 64]
            bv_bc = consts.tile([128, NH_CORE, D], F32)
            nc.sync.dma_start(
                bv_bc[:],
                bv_d.ap().rearrange("(h d) -> h d", d=D).partition_broadcast(128),
            )

            # --- big persistent tiles ----------------------------------
            xt = big.tile([128, JT, S], F32R)            # X^T
            wt_q = big.tile([128, JT, DC], F32R)         # Wq^T
            wt_k = big.tile([128, JT, DC], F32R)
            wt_v = big.tile([128, JT, DC], F32R)
            qt = big.tile([128, 2, S], F32R)             # QT (hp-packed), pre-scaled
            kt_sb = big.tile([128, 2, S], F32R)          # KT
            v_sb = big.tile([128, ST_T, NH_CORE, D + 1], F32R)  # V + ones col

            ones_c = consts.tile([128, NH_CORE], F32)
            nc.vector.memset(ones_c[:], 1.0)
            for st in range(ST_T):
                nc.vector.tensor_copy(v_sb[:, st, :, D], ones_c[:])

            # --- phase A: transposes of X and W ------------------------
            with tc.tile_pool(name="stage", bufs=3) as stage, \
                 tc.tile_pool(name="ps_t", bufs=4, space="PSUM") as ps_t:
                for st in range(ST_T):
                    xs = stage.tile([128, H], F32, tag="xs")
                    nc.sync.dma_start(xs[:], x_d.ap()[st * 128 : (st + 1) * 128, :])
                    for jt in range(JT):
                        pt = ps_t.tile([128, 128], F32, tag="pt")
                        nc.tensor.transpose(
                            pt[:], xs[:, jt * 128 : (jt + 1) * 128], ident[:]
                        )
                        nc.vector.tensor_copy(
                            xt[:, jt, st * 128 : (st + 1) * 128], pt[:]
                        )
                for w_d, wt in ((wq_d, wt_q), (wk_d, wt_k), (wv_d, wt_v)):
                    for d2 in range(2):
                        ws = stage.tile([128, H], F32, tag="ws")
                        nc.sync.dma_start(
                            ws[:], w_d.ap()[d2 * 128 : (d2 + 1) * 128, :]
                        )
                        for jt in range(JT):
                            pt = ps_t.tile([128, 128], F32, tag="pt")
                            nc.tensor.transpose(
                                pt[:], ws[:, jt * 128 : (jt + 1) * 128], ident[:]
                            )
                            nc.vector.tensor_copy(
                                wt[:, jt, d2 * 128 : (d2 + 1) * 128], pt[:]
                            )

            # --- phase B: projections ----------------------------------
            with tc.tile_pool(name="ps_p", bufs=2, space="PSUM") as ps_p:
                # QT / KT: [128 d (2 heads), S], psum over j-tiles
                for wt, dst, b_sb, scl in (
                    (wt_q, qt, bq_sb, SCALE),
                    (wt_k, kt_sb, bk_sb, 1.0),
                ):
                    for hp in range(2):
                        for qc in range(QC):
                            pq = ps_p.tile([128, 512], F32, tag="pq")
                            for jt in range(JT):
                                nc.tensor.matmul(
                                    pq[:],
                                    wt[:, jt, hp * 128 : (hp + 1) * 128],
                                    xt[:, jt, qc * 512 : (qc + 1) * 512],
                                    start=(jt == 0),
                                    stop=(jt == JT - 1),
                                )
                            nc.scalar.activation(
                                dst[:, hp, qc * 512 : (qc + 1) * 512],
                                pq[:],
                                AF.Identity,
                                bias=b_sb[:, hp : hp + 1],
                                scale=scl,
                            )
                # V natural: [s-tile 128, 256], psum over j-tiles
                for st in range(ST_T):
                    pv = ps_p.tile([128, DC], F32, tag="pv")
                    for jt in range(JT):
                        nc.tensor.matmul(
                            pv[:],
                            xt[:, jt, st * 128 : (st + 1) * 128],
                            wt_v[:, jt, :],
                            start=(jt == 0),
                            stop=(jt == JT - 1),
                        )
                    nc.vector.tensor_tensor(
                        v_sb[:, st, :, 0:D],
                        pv[:].rearrange("p (h d) -> p h d", d=D),
                        bv_bc[:],
                        ALU.add,
                    )

            # --- phase C: attention ------------------------------------
            with tc.tile_pool(name="att", bufs=3) as att, \
                 tc.tile_pool(name="ps_s", bufs=2, space="PSUM") as ps_s, \
                 tc.tile_pool(name="ps_c", bufs=1, space="PSUM") as ps_c, \
                 tc.tile_pool(name="ps_o", bufs=2, space="PSUM") as ps_o, \
                 tc.tile_pool(name="fin", bufs=4) as fin:
                for hp in range(2):
                    for qc in range(QC):
                        qsl = slice(qc * 512, (qc + 1) * 512)
                        pc = [
                            ps_c.tile([D + 1, 512], F32, tag=f"pc{h2}",
                                      name=f"pc{h2}_{hp}_{qc}")
                            for h2 in range(2)
                        ]
                        for kt in range(KT_T):
                            rts = []
                            for h2 in range(2):
                                dsl = slice(h2 * 64, (h2 + 1) * 64)
                                ps = ps_s.tile([128, 512], F32, tag=f"ps{h2}")
                                nc.tensor.matmul(
                                    ps[:],
                                    kt_sb[dsl, hp, kt * 128 : (kt + 1) * 128],
                                    qt[dsl, hp, qsl],
                                    start=True,
                                    stop=True,
                                )
                                rt = att.tile([128, 512], F32R, tag=f"rt{h2}")
                                if h2 == 0:
                                    nc.scalar.activation(
                                        rt[:], ps[:], AF.Relu,
                                        bias=mask_sb[:, kt : kt + 1],
                                    )
                                else:
                                    nc.vector.tensor_scalar(
                                        rt[:], ps[:],
                                        mask_sb[:, kt : kt + 1], 0.0,
                                        ALU.add, ALU.max,
                                    )
                                rts.append(rt)
                            for h2 in range(2):
                                hh = hp * 2 + h2
                                nc.tensor.matmul(
                                    pc[h2][:],
                                    v_sb[:, kt, hh, :],
                                    rts[h2][:],
                                    start=(kt == 0),
                                    stop=(kt == KT_T - 1),
                                )
                        # epilogue: transpose back, normalize, store
                        for h2 in range(2):
                            hh = hp * 2 + h2
                            cu = att.tile([D + 1, 512], F32, tag="cu")
                            nc.vector.tensor_copy(cu[:], pc[h2][:])
                            for i in range(4):
                                po = ps_o.tile([128, D + 1], F32, tag="po")
                                nc.tensor.transpose(
                                    po[:],
                                    cu[:, i * 128 : (i + 1) * 128],
                                    ident[: D + 1, : D + 1],
                                )
                                den = fin.tile([128, 1], F32, tag="den")
                                nc.vector.tensor_scalar_add(
                                    den[:], po[:, D : D + 1], EPS
                                )
                                rec = fin.tile([128, 1], F32, tag="rec")
                                nc.vector.reciprocal(rec[:], den[:])
                                ob = fin.tile([128, D], F32, tag="ob")
                                nc.vector.tensor_scalar(
                                    ob[:], po[:, 0:D], rec[:], None, ALU.mult
                                )
                                nc.sync.dma_start(
                                    out_d.ap()[
                                        qc * 512 + i * 128 : qc * 512 + (i + 1) * 128,
                                        hh * D : (hh + 1) * D,
                                    ],
                                    ob[:],
                                )
    nc.compile()
    return nc


def kernel(hidden_states, attention_mask, Wq, bq, Wk, bk, Wv, bv):
    if "nc" not in _CACHE:
        _CACHE["nc"] = _build()
    nc = _CACHE["nc"]

    x = np.ascontiguousarray(np.asarray(hidden_states, dtype=np.float32))
    mask = np.asarray(attention_mask, dtype=np.float32).reshape(B, S)
    ws = {
        "wq": np.asarray(Wq, dtype=np.float32),
        "wk": np.asarray(Wk, dtype=np.float32),
        "wv": np.asarray(Wv, dtype=np.float32),
    }
    bs = {
        "bq": np.asarray(bq, dtype=np.float32),
        "bk": np.asarray(bk, dtype=np.float32),
        "bv": np.asarray(bv, dtype=np.float32),
    }

    in_maps = []
    for c in range(8):
        b, hg = c // 4, c % 4
        rs = slice(hg * DC, (hg + 1) * DC)
        im = {"x": x[b], "mask": np.ascontiguousarray(mask[b])}
        for k, w in ws.items():
            im[k] = np.ascontiguousarray(w[rs])
        for k, v in bs.items():
            im[k] = np.ascontiguousarray(v[rs])
        in_maps.append(im)

    res = bass_utils.run_bass_kernel_spmd(nc, in_maps, core_ids=list(range(8)))

    out = np.empty((B, S, H), dtype=np.float32)
    for c in range(8):
        b, hg = c // 4, c % 4
        out[b, :, hg * DC : (hg + 1) * DC] = res.results[c]["out"]
    return out
